# revision 1
# baseline (speedup 1.0000x reference)
"""Trainium2 kernel v5: mixed-precision (fp8e4 + bf16) streaming max-pool.

out[n] = (1/32768) * sum_{c,blocks} maxpool3d_2x2x2(o[n]) + sum_c bias[c]

The kernel is DMA-bound (360 GB/s modeled); max-pooling commutes with
monotone rounding, so inputs upload in reduced precision (end-to-end rel err
~1e-3 vs the 2e-2 gate). Host permutes each (n, c, pd) row of 8192 values to
[h2(32 groups), wp(2), hp(2), dp(2), w2(32)] so each max-tree level is a
packed contiguous-halves TensorTensor on DVE:
    L1 (wp): [*,256]g -> [*,128]g   L2 (hp): -> [*,64]g   L3 (dp): -> [*,32]g

Only DVE can run TensorTensor (the Pool/gpsimd engine fails the hardware ISA
check, and ACT has no binary ops), so the dtype split balances DVE against
the stream: bf16 groups run at DVE's 2x packed rate, fp8 groups halve their
DMA bytes but run at 1x. nB=22 bf16 / 10 fp8 puts DVE busy (~4.83us/tile)
just under the per-tile stream time (~4.91us).

Block sums ride PE matmuls with a SCALE-valued bf16 ones vector into PSUM
(bank0 = m3 cols 0:512, bank1 = 512:1024), accumulated across each batch's 4
tiles. Finish reductions ride the idle ACT engine (Copy+accum). Batch 0
completes mid-stream at tile 3. Bank1 of batch 1 stops at tile 6; tile 7's
bank1 contribution goes through fused stt accumulators so the stream ends on
a tiny 2-group bf16 chain -> [1,2] matmul -> small DVE reduces -> store.
"""

import numpy as np

N, C, D, H, W = 16, 32, 32, 64, 64
N_CORES = 8
N_PER_CORE = N // N_CORES          # 2
PD = D // 2                        # 16
ROWS = N_PER_CORE * C * PD         # 1024
P = 128                            # SBUF partitions
N_TILES = ROWS // P                # 8
TILES_PER_N = N_TILES // N_PER_CORE  # 4

NG = 32                            # groups per row (= h2)
GW = 256                           # values per group
NB = 16                            # bf16 groups (16..31)
GD = NG - NB                       # fp8 groups (0..GD), upcast on ACT
FCOLS = GD * GW                    # fp8 cols per row
BCOLS = NB * GW                    # bf16 cols per row
NBA = 9                           # bf16 groups in first chunk (10..20)
NBB = NB - NBA                     # bf16 groups in second chunk (21..31)
SCALE = 1.0 / (2.0 * PD * (H // 2) * (W // 2))  # 1/32768, exact in bf16

_NC_CACHE = None


def _build_nc():
    import concourse.bacc as bacc
    import concourse.tile as tile
    import concourse.mybir as mybir

    f32 = mybir.dt.float32
    bf16 = mybir.dt.bfloat16
    f8 = mybir.dt.float8e4
    COPY = mybir.ActivationFunctionType.Copy
    nc = bacc.Bacc("TRN2", target_bir_lowering=False, debug=False)

    xf_in = nc.dram_tensor("xf", [ROWS, FCOLS], f8, kind="ExternalInput")
    xb_in = nc.dram_tensor("xb", [ROWS, BCOLS], bf16, kind="ExternalInput")
    b_in = nc.dram_tensor("bias", [1, C], f32, kind="ExternalInput")
    out_d = nc.dram_tensor("out", [1, N_PER_CORE], f32, kind="ExternalOutput")

    with tile.TileContext(nc) as tc:
        with (
            tc.tile_pool(name="xf", bufs=4) as xfp,
            tc.tile_pool(name="xb", bufs=4) as xbp,
            tc.tile_pool(name="xc", bufs=3) as xcp,
            tc.tile_pool(name="m1", bufs=3) as m1p,
            tc.tile_pool(name="m2", bufs=3) as m2p,
            tc.tile_pool(name="m3", bufs=3) as m3p,
            tc.tile_pool(name="misc", bufs=1) as misc,
            tc.tile_pool(name="ps", bufs=1, space="PSUM") as pp,
        ):
            ones = misc.tile([P, 1], bf16)
            nc.vector.memset(ones[:], SCALE)
            onesf = misc.tile([P, 1], f32)
            nc.vector.memset(onesf[:], SCALE)
            # bias on the ACT ring; ACT also reduces it into the partial rows
            bt = misc.tile([1, C], f32)
            nc.scalar.dma_start(bt[:], b_in[:])
            bscr = misc.tile([1, C], f32)
            # partial-sum rows per batch: [r_bank0, r_bank1, r_extra, bsum]
            r0 = misc.tile([1, 4], f32)
            r1 = misc.tile([1, 4], f32)
            nc.scalar.activation(bscr[:], bt[:], COPY, accum_out=r0[:, 3:4])
            nc.scalar.activation(bscr[:], bt[:], COPY, accum_out=r1[:, 3:4])
            nc.vector.memset(r0[:, 2:3], 0.0)

            ps = [
                [
                    pp.tile([1, 512], f32, name=f"ps{ni}_{bi}", tag=f"ps{ni}_{bi}")
                    for bi in range(2)
                ]
                for ni in range(N_PER_CORE)
            ]
            psT = pp.tile([1, 1], f32)
            accT = misc.tile([P, 1], f32)
            fin = misc.tile([1, N_PER_CORE], f32)
            scr0 = misc.tile([1, 512], f32)
            scr1 = misc.tile([1, 512], f32)

            def l1(src, g0, ng):
                v = src.rearrange("p (g w) -> p g w", w=GW)
                nc.vector.tensor_max(
                    m1v[:, g0 : g0 + ng, :], v[:, :, 0:128], v[:, :, 128:256]
                )

            def l2(g0, ng):
                m1h = m1[:].rearrange("p (g h w) -> p g h w", h=2, w=64)
                nc.vector.tensor_max(
                    m2v[:, g0 : g0 + ng, :],
                    m1h[:, g0 : g0 + ng, 0, :],
                    m1h[:, g0 : g0 + ng, 1, :],
                )

            def l3(g0, ng):
                m2h = m2[:].rearrange("p (g h w) -> p g h w", h=2, w=32)
                nc.vector.tensor_max(
                    m3v[:, g0 : g0 + ng, :],
                    m2h[:, g0 : g0 + ng, 0, :],
                    m2h[:, g0 : g0 + ng, 1, :],
                )

            def l3_acc(g0, ng, col):
                # L3 max fused with a free-axis sum into accT[:, col]
                m2h = m2[:].rearrange("p (g h w) -> p g h w", h=2, w=32)
                nc.vector.scalar_tensor_tensor(
                    out=m3v[:, g0 : g0 + ng, :],
                    in0=m2h[:, g0 : g0 + ng, 0, :],
                    scalar=0.0,
                    in1=m2h[:, g0 : g0 + ng, 1, :],
                    op0=mybir.AluOpType.bypass,
                    op1=mybir.AluOpType.max,
                    accum_out=accT[:, col : col + 1],
                )

            # software-pipelined fp8 stream: tile t's fp8 chunk lands
            # during tile t-1's window so the ACT upcasts run back-to-back
            # and finish before the stream ends
            xft_cur = xfp.tile([P, FCOLS], f8, tag="xf0")
            nc.sync.dma_start(xft_cur[:], xf_in[0:P, :])
            for t in range(N_TILES):
                rows = slice(P * t, P * (t + 1))
                n_idx = t // TILES_PER_N
                start = t % TILES_PER_N == 0
                last = t == N_TILES - 1

                xft = xft_cur
                if not last:
                    rows_n = slice(P * (t + 1), P * (t + 2))
                    xft_cur = xfp.tile([P, FCOLS], f8, tag="xf")
                    nc.sync.dma_start(xft_cur[:], xf_in[rows_n, :])
                xba = xbp.tile([P, NBA * GW], bf16, tag="xba")
                nc.sync.dma_start(xba[:], xb_in[rows, 0 : NBA * GW])
                xbb = xbp.tile([P, NBB * GW], bf16, tag="xbb")
                nc.sync.dma_start(xbb[:], xb_in[rows, NBA * GW :])

                m1 = m1p.tile([P, NG * 128], bf16, tag="m1")
                m1v = m1[:].rearrange("p (g w) -> p g w", w=128)
                m2 = m2p.tile([P, NG * 64], bf16, tag="m2")
                m2v = m2[:].rearrange("p (g w) -> p g w", w=64)
                m3 = m3p.tile([P, NG * 32], bf16, tag="m3")
                m3v = m3[:].rearrange("p (g w) -> p g w", w=32)

                # --- ACT upcasts the fp8 chunk; DVE runs all L1 at 2x ---
                xc = xcp.tile([P, GD * GW], bf16, tag="xc")
                nc.scalar.activation(xc[:], xft[:], COPY)
                l1(xc[:], 0, GD)
                l1(xba[:], GD, NBA)
                if not last:
                    l1(xbb[:], GD + NBA, NBB)
                    l2(0, NG)
                    l3(0, NG)
                    nc.tensor.matmul(ps[n_idx][0][:], ones[:], m3[:, 0:512],
                                     start=start, stop=t in (3, 7))
                    nc.tensor.matmul(ps[n_idx][1][:], ones[:], m3[:, 512:1024],
                                     start=start, stop=t in (3, 6))
                    if t == 4:
                        # finish batch 0 (emitted here so the ACT queue
                        # reaches tile 4's casts before these psum waits)
                        nc.scalar.activation(scr0[:], ps[0][0][:], COPY,
                                             accum_out=r0[:, 0:1])
                        nc.scalar.activation(scr1[:], ps[0][1][:], COPY,
                                             accum_out=r0[:, 1:2])
                        nc.vector.reduce_sum(fin[:, 0:1], r0[:],
                                             axis=mybir.AxisListType.X)
                        nc.gpsimd.dma_start(out_d[:, 0:1], fin[:, 0:1])
                else:
                    # bank1 of batch 1 completed at tile 6
                    nc.scalar.activation(scr1[:], ps[1][1][:], COPY,
                                         accum_out=r1[:, 1:2])
                    # tile 7: bank0 via psum + ACT reduce; bank1 via fused stt
                    l2(0, 16)
                    l3(0, 16)
                    nc.tensor.matmul(ps[1][0][:], ones[:], m3[:, 0:512],
                                     start=False, stop=True)
                    nc.scalar.activation(scr0[:], ps[1][0][:], COPY,
                                         accum_out=r1[:, 0:1])
                    l1(xbb[:], GD + NBA, NBB)
                    l2(16, 16)
                    l3_acc(16, 16, 0)
                    nc.tensor.matmul(psT[:], onesf[:], accT[:],
                                     start=True, stop=True)
                    nc.vector.reduce_sum(r1[:, 2:3], psT[:],
                                         axis=mybir.AxisListType.X)
                    nc.vector.reduce_sum(fin[:, 1:2], r1[:],
                                         axis=mybir.AxisListType.X)
                    nc.sync.dma_start(out_d[:, 1:2], fin[:, 1:2])

    nc.compile()
    return nc


_RUNNER_CACHE = None


def _build_runner(nc):
    """Jitted shard_map runner built once; per call only input upload +
    execution happen."""
    import jax
    import numpy as _np
    from jax.sharding import Mesh, PartitionSpec, NamedSharding
    from concourse import bass2jax
    import concourse.mybir as mybir

    bass2jax.install_neuronx_cc_hook()
    partition_name = nc.partition_id_tensor.name if nc.partition_id_tensor else None
    in_names, out_names, out_avals, zero_outs = [], [], [], []
    for alloc in nc.m.functions[0].allocations:
        if not isinstance(alloc, mybir.MemoryLocationSet):
            continue
        name = alloc.memorylocations[0].name
        if alloc.kind == "ExternalInput":
            if name != partition_name:
                in_names.append(name)
        elif alloc.kind == "ExternalOutput":
            out_names.append(name)
            shape = tuple(alloc.tensor_shape)
            dtype = mybir.dt.np(alloc.dtype)
            out_avals.append(jax.core.ShapedArray(shape, dtype))
            zero_outs.append(_np.zeros(shape, dtype))
    n_params = len(in_names)
    n_outs = len(out_avals)
    all_in = list(in_names) + list(out_names)
    if partition_name is not None:
        all_in.append(partition_name)

    def _body(*args):
        operands = list(args)
        if partition_name is not None:
            operands.append(bass2jax.partition_id_tensor())
        return tuple(
            bass2jax._bass_exec_p.bind(
                *operands,
                out_avals=tuple(out_avals),
                in_names=tuple(all_in),
                out_names=tuple(out_names),
                lowering_input_output_aliases=(),
                sim_require_finite=True,
                sim_require_nnan=True,
                nc=nc,
            )
        )

    devices = jax.devices()[:N_CORES]
    mesh = Mesh(_np.asarray(devices), ("core",))
    n_tot = n_params + n_outs
    fn = jax.jit(
        jax.shard_map(
            _body,
            mesh=mesh,
            in_specs=(PartitionSpec("core"),) * n_tot,
            out_specs=(PartitionSpec("core"),) * n_outs,
            check_vma=False,
        ),
        donate_argnums=tuple(range(n_params, n_tot)),
        keep_unused=True,
    )
    sharding = NamedSharding(mesh, PartitionSpec("core"))

    def run(concat_inputs_by_name):
        dev_in = [
            jax.device_put(concat_inputs_by_name[nm], sharding) for nm in in_names
        ]
        zs = [
            jax.device_put(
                _np.zeros((N_CORES * z.shape[0],) + z.shape[1:], z.dtype), sharding
            )
            for z in zero_outs
        ]
        outs = fn(*dev_in, *zs)
        return {
            name: _np.asarray(outs[i]).reshape(N_CORES, *out_avals[i].shape)
            for i, name in enumerate(out_names)
        }

    return run


def _host_pack(o):
    """Permute rows to [h2, wp, hp, dp, w2] and dtype-split the groups."""
    import ml_dtypes

    v = np.ascontiguousarray(o, dtype=np.float32).reshape(
        N, C, PD, 2, 32, 2, 32, 2
    )  # n c pd dp h2 hp w2 wp
    v = v.transpose(0, 1, 2, 4, 7, 5, 3, 6)  # n c pd h2 wp hp dp w2
    rows = v.reshape(N_CORES * ROWS, NG * GW)
    xf = rows[:, :FCOLS].astype(ml_dtypes.float8_e4m3)
    xb = rows[:, FCOLS:].astype(ml_dtypes.bfloat16)
    return np.ascontiguousarray(xf), np.ascontiguousarray(xb)


def kernel(o: np.ndarray, bias: np.ndarray) -> np.ndarray:
    global _NC_CACHE, _RUNNER_CACHE

    if _NC_CACHE is None:
        _NC_CACHE = _build_nc()
    nc = _NC_CACHE

    xf, xb = _host_pack(o)
    b2 = np.ascontiguousarray(bias, dtype=np.float32).reshape(1, C)
    b_rep = np.ascontiguousarray(
        np.broadcast_to(b2, (N_CORES, C)).reshape(N_CORES * 1, C)
    )

    try:
        if _RUNNER_CACHE is None:
            _RUNNER_CACHE = _build_runner(nc)
        res = _RUNNER_CACHE({"xf": xf, "xb": xb, "bias": b_rep})
        out = res["out"].reshape(N_CORES * N_PER_CORE)
    except Exception:
        from concourse.bass_utils import run_bass_kernel_spmd

        in_maps = [
            {
                "xf": xf[ROWS * k : ROWS * (k + 1)],
                "xb": xb[ROWS * k : ROWS * (k + 1)],
                "bias": b2,
            }
            for k in range(N_CORES)
        ]
        r = run_bass_kernel_spmd(nc, in_maps, core_ids=list(range(N_CORES)))
        out = np.concatenate(
            [r.results[k]["out"].reshape(N_PER_CORE) for k in range(N_CORES)]
        )
    return out.reshape(N, 1, 1, 1).astype(np.float32)

